# revision 1
# baseline (speedup 1.0000x reference)
"""Causal multi-head self-attention on 8 Trainium2 NeuronCores.

Sharding: tensor-parallel over heads. 16 heads / 8 cores = 2 heads per core.
Each core computes the QKV projection for its 2 heads (full sequence, both
batches), causal flash-style attention for its 2 heads, and a partial output
projection against its slice of W_o columns. The host sums the 8 partial
outputs (the "all-reduce" of the tensor-parallel scheme, done during unshard).

Matmul inputs are fp16 (PE streams 1 row/cycle vs 4 for fp32; fp16 keeps
11 mantissa bits vs bf16's 8), accumulation is always fp32 in PSUM, softmax
runs in fp32. End-to-end error vs the fp32 reference is ~4e-4 relative.

Device layout (contraction dim always on partitions):
  - x passed pre-transposed and pre-cast: xT [D, B*S] fp16.
  - Projection computes Q^T/K^T/V^T [128=2*dk, S] per batch directly.
  - Scores computed transposed, S^T[k, q] = K^T.T @ Q^T (fp32 PSUM), both
    heads into one [128, 2, 512] PSUM tile via separate PE row groups (the
    two matmuls run concurrently in different PE row strips).
  - One ACT exp per score tile (PSUM -> SBUF fp16), causal diagonal blocks
    column-sliced, the remaining 128-band masked with a triangular multiply.
  - V^T transposed on-PE to V[tok, dv] with a ones column appended, so the
    AV matmul also accumulates the softmax row-sums (row 64 of the output).
  - Normalization: stage O^T/row-sum to SBUF (frees PSUM), GPSIMD
    partition-broadcast of the row-sum (base-0 output only: HW ignores the
    out AP base), ~51ULP reciprocal, DVE multiply into mhaT fp16.
  - Output projection: out[tok,:] = mhaT_tile.T @ WoT, fp32 result to DRAM.

The emission order software-pipelines the batches: batch1's projection is
interleaved into batch0's attention (attention is exp/ACT-gated, leaving PE
slack), and batch0's output projection into batch1's attention.
"""

import numpy as np

import concourse.bacc as bacc
import concourse.mybir as mybir
import concourse.tile as tile

FP32 = mybir.dt.float32
FP16 = mybir.dt.float16

B = 2
S = 2048
D = 1024
NUM_HEADS = 16
DK = 64
NCORES = 8
HPC = NUM_HEADS // NCORES  # heads per core = 2
HD = HPC * DK  # 128, head dims per core

QCW = 512  # q chunk width
KTW = 128  # k tile width (partition dim)

NP_IN = np.float16


def build_nc(d=D, s=S, b=B):
    """Build the per-core Bass program. All 8 cores run this same program."""
    assert d % 128 == 0 and s % QCW == 0 and QCW % KTW == 0
    ndc = d // 128  # d_model chunks
    nqc = s // QCW  # q chunks per batch
    nkt = s // KTW  # k tiles per batch
    kpq = QCW // KTW  # k tiles per q chunk (4)
    ntt = s // 128  # token tiles per batch

    nc = bacc.Bacc("TRN2", target_bir_lowering=False)

    xT_d = nc.dram_tensor("xT", [d, b * s], FP16, kind="ExternalInput")
    wt_d = nc.dram_tensor("wqkvT", [d, 3 * HD], FP16, kind="ExternalInput")
    wo_d = nc.dram_tensor("woT", [HD, d], FP16, kind="ExternalInput")
    tri_d = nc.dram_tensor("tri", [128, 128], FP16, kind="ExternalInput")
    id_d = nc.dram_tensor("ident", [128, 128], FP16, kind="ExternalInput")
    out_d = nc.dram_tensor("out", [b * s, d], FP32, kind="ExternalOutput")

    with tile.TileContext(nc) as tc:
        with (
            tc.tile_pool(name="consts", bufs=1) as consts,
            tc.tile_pool(name="xts", bufs=b * ndc) as xts_pool,
            tc.tile_pool(name="qkv", bufs=2) as qkv_pool,
            tc.tile_pool(name="vsb", bufs=2) as v_pool,
            tc.tile_pool(name="pt", bufs=4) as pt_pool,
            tc.tile_pool(name="mha", bufs=2) as mha_pool,
            tc.tile_pool(name="osb", bufs=3) as out_pool,
            tc.tile_pool(name="small", bufs=2) as small_pool,
            tc.tile_pool(name="ps_mm", bufs=2, space="PSUM") as ps_mm,
            tc.tile_pool(name="ps_s", bufs=2, space="PSUM") as ps_s,
            tc.tile_pool(name="ps_o", bufs=1, space="PSUM") as ps_o,
        ):
            # ---- input loads: weights for the first projection, then x of
            # batch 0, then the small attention constants, then x of batch 1
            wt_sb = consts.tile([128, ndc, 3 * HD], FP16)
            for k in range(ndc):
                nc.sync.dma_start(wt_sb[:, k, :], wt_d[128 * k : 128 * (k + 1), :])
            xts_all = []
            for bi in range(b):
                xts_all.append(
                    [
                        xts_pool.tile([128, s], FP16, name=f"xt{bi}_{k}", tag="xt")
                        for k in range(ndc)
                    ]
                )
            for k in range(ndc):
                nc.sync.dma_start(
                    xts_all[0][k], xT_d[128 * k : 128 * (k + 1), 0:s]
                )
            tri_sb = consts.tile([128, 128], FP16)
            nc.sync.dma_start(tri_sb, tri_d[:, :])
            id_sb = consts.tile([128, 128], FP16)
            nc.sync.dma_start(id_sb, id_d[:, :])
            wo_sb = consts.tile([128, d], FP16)
            nc.sync.dma_start(wo_sb, wo_d[:, :])
            for bi in range(1, b):
                for k in range(ndc):
                    nc.sync.dma_start(
                        xts_all[bi][k],
                        xT_d[128 * k : 128 * (k + 1), bi * s : (bi + 1) * s],
                    )

            qkvTs = [qkv_pool.tile([128, 3, s], FP16, name=f"qkvT{bi}", tag="qkvT")
                     for bi in range(b)]
            v_sbs = [v_pool.tile([128, nkt, 2 * (DK + 1)], FP16, name=f"v{bi}",
                                 tag="vsb") for bi in range(b)]
            mhaTs = [mha_pool.tile([128, s], FP16, name=f"mhaT{bi}", tag="mhaT")
                     for bi in range(b)]

            def emit_proj_group(bi, m, n):
                qkvT, xts = qkvTs[bi], xts_all[bi]
                pp = ps_mm.tile([128, QCW], FP32, name="pp", tag="mm")
                for k in range(ndc):
                    nc.tensor.matmul(
                        pp,
                        wt_sb[:, k, 128 * m : 128 * (m + 1)],
                        xts[k][:, QCW * n : QCW * (n + 1)],
                        start=(k == 0),
                        stop=(k == ndc - 1),
                    )
                # ACT is otherwise idle during projection phases; it also
                # casts fp32 PSUM -> fp16 SBUF on the way out.
                nc.scalar.copy(qkvT[:, m, QCW * n : QCW * (n + 1)], pp)

            def emit_vsb_init(bi):
                nc.gpsimd.memset(v_sbs[bi], 1.0)

            def emit_trans(bi, t):
                qkvT, v_sb = qkvTs[bi], v_sbs[bi]
                tp = ps_mm.tile([128, 128], FP16, name="tp", tag="mm")
                nc.tensor.transpose(tp, qkvT[:, 2, 128 * t : 128 * (t + 1)], id_sb)
                nc.vector.tensor_copy(v_sb[:, t, 0:DK], tp[:, 0:DK])
                nc.vector.tensor_copy(
                    v_sb[:, t, DK + 1 : 2 * DK + 1], tp[:, DK : 2 * DK]
                )

            def emit_attn_chunk(bi, qc, fillers):
                """One q-chunk of attention for batch bi.

                The AV matmuls lag the score/exp stream by LAG blocks so the
                first AV (which must wait for the previous chunk's PSUM
                accumulator to free) never head-of-line-blocks the in-order
                PE queue that feeds ACT. `fillers` (independent PE work, e.g.
                the previous chunk's out-projection tiles) drain one per
                block in the lagged slot.
                """
                qkvT, v_sb, mhaT = qkvTs[bi], v_sbs[bi], mhaTs[bi]
                q0 = QCW * qc
                oA = ps_o.tile([DK + 1, QCW], FP32, name="oA", tag="oA")
                oB = ps_o.tile([DK + 1, QCW], FP32, name="oB", tag="oB")
                kts = kpq * (qc + 1)
                LAG = 2
                pts = {}
                fillers = list(fillers)
                for i in range(kts + LAG):
                    if i < kts:
                        kt = i
                        c0 = KTW * (kt - kpq * qc) if kt >= kpq * qc else 0
                        sp = ps_s.tile([128, 2, QCW], FP32, name="sp", tag="s")
                        # scores S^T[k, q]; heads in separate PE row groups
                        nc.tensor.matmul(
                            sp[:, 0, c0:QCW],
                            qkvT[0:DK, 1, KTW * kt : KTW * (kt + 1)],
                            qkvT[0:DK, 0, q0 + c0 : q0 + QCW],
                        )
                        nc.tensor.matmul(
                            sp[:, 1, c0:QCW],
                            qkvT[DK : 2 * DK, 1, KTW * kt : KTW * (kt + 1)],
                            qkvT[DK : 2 * DK, 0, q0 + c0 : q0 + QCW],
                        )
                        pt = pt_pool.tile(
                            [128, 2, QCW], FP16, name="pt", tag="pt"
                        )
                        nc.scalar.activation(
                            pt[:, :, c0:QCW],
                            sp[:, :, c0:QCW],
                            mybir.ActivationFunctionType.Exp,
                        )
                        if kt >= kpq * qc:
                            # triangular mask on the diagonal 128-band
                            nc.vector.tensor_mul(
                                pt[:, 0, c0 : c0 + KTW],
                                pt[:, 0, c0 : c0 + KTW],
                                tri_sb,
                            )
                            nc.vector.tensor_mul(
                                pt[:, 1, c0 : c0 + KTW],
                                pt[:, 1, c0 : c0 + KTW],
                                tri_sb,
                            )
                        pts[kt] = (pt, c0)
                    if i >= LAG:
                        kt = i - LAG
                        pt, c0 = pts.pop(kt)
                        nc.tensor.matmul(
                            oA[:, c0:QCW],
                            v_sb[:, kt, 0 : DK + 1],
                            pt[:, 0, c0:QCW],
                            start=(kt == 0),
                            stop=(kt == kts - 1),
                        )
                        nc.tensor.matmul(
                            oB[:, c0:QCW],
                            v_sb[:, kt, DK + 1 : 2 * DK + 2],
                            pt[:, 1, c0:QCW],
                            start=(kt == 0),
                            stop=(kt == kts - 1),
                        )
                        if fillers:
                            fillers.pop(0)()
                # normalize: stage O^T + row-sum to base-0 SBUF (frees PSUM),
                # broadcast row-sum (base-0 out only), reciprocal, multiply
                for h, oh in ((0, oA), (1, oB)):
                    ost = small_pool.tile([DK, QCW], FP32, name="ost", tag=f"ost{h}")
                    nc.vector.tensor_copy(ost, oh[0:DK, :])
                    t = small_pool.tile([1, QCW], FP32, name="t", tag=f"t{h}")
                    nc.vector.tensor_copy(t, oh[DK : DK + 1, :])
                    bc = small_pool.tile([DK, QCW], FP32, name="bc", tag=f"bc{h}")
                    nc.gpsimd.partition_broadcast(bc, t, channels=DK)
                    nc.vector.reciprocal_approx_fast(out=bc, in_=bc)
                    nc.vector.tensor_mul(
                        mhaT[DK * h : DK * (h + 1), q0 : q0 + QCW], ost, bc
                    )

            def emit_fp_tile(bi, t):
                mhaT = mhaTs[bi]
                fps = []
                for half in range(d // QCW):
                    fp = ps_mm.tile([128, QCW], FP32, name="fp", tag="mm")
                    nc.tensor.matmul(
                        fp,
                        mhaT[:, 128 * t : 128 * (t + 1)],
                        wo_sb[:, QCW * half : QCW * (half + 1)],
                    )
                    fps.append(fp)
                ob = out_pool.tile([128, d], FP32, name="ob", tag="ob")
                for half in range(d // QCW):
                    nc.vector.tensor_copy(
                        ob[:, QCW * half : QCW * (half + 1)], fps[half]
                    )
                r0 = bi * s + 128 * t
                nc.sync.dma_start(out_d[r0 : r0 + 128, :], ob)

            # ---- per batch: projection + V transpose (PE-dense), then
            # attention with the batch's own out-projection tiles emitted
            # right after each q-chunk normalizes (fills the PE slack of the
            # exp/ACT-bound attention phase; aux PSUM slots are free then)
            tpq = ntt // nqc  # out-proj token tiles ready per q-chunk
            for bi in range(b):
                for m in range(3):
                    for n in range(nqc):
                        emit_proj_group(bi, m, n)
                emit_vsb_init(bi)
                for t in range(nkt):
                    emit_trans(bi, t)
                prev_fp = []
                for qc in range(nqc):
                    emit_attn_chunk(bi, qc, prev_fp)
                    prev_fp = [
                        (lambda t=t: emit_fp_tile(bi, t))
                        for t in range(tpq * qc, tpq * (qc + 1))
                    ]
                for th in prev_fp:
                    th()

    nc.compile()
    return nc


def make_core_inputs(x, W_qkv, W_o, d=D, s=S, b=B):
    """Host-side shard prep. Returns list of per-core input dicts."""
    nh = W_qkv.shape[0] // (3 * DK)
    xT = np.ascontiguousarray(
        x.astype(np.float32).transpose(2, 0, 1).reshape(d, b * s).astype(NP_IN)
    )
    tri = np.triu(np.ones((128, 128), dtype=NP_IN))  # tri[k,q]=1 iff q>=k
    ident = np.eye(128, dtype=NP_IN)
    scale = np.float32(1.0 / np.sqrt(DK))
    in_maps = []
    for c in range(NCORES):
        h0 = HPC * c
        r = slice(h0 * DK, (h0 + HPC) * DK)
        wq = W_qkv[0 * nh * DK :][r] * scale
        wk = W_qkv[1 * nh * DK :][r]
        wv = W_qkv[2 * nh * DK :][r]
        ws = np.concatenate([wq, wk, wv], axis=0)  # [3*HD, d]
        wT = np.ascontiguousarray(ws.T.astype(NP_IN))  # [d, 3*HD]
        woT = np.ascontiguousarray(W_o[:, r].T.astype(NP_IN))  # [HD, d]
        in_maps.append(
            {"xT": xT, "wqkvT": wT, "woT": woT, "tri": tri, "ident": ident}
        )
    return in_maps


_NC_CACHE = {}


def kernel(x, W_qkv, W_o):
    from concourse.bass_utils import run_bass_kernel_spmd

    b, s, d = x.shape
    if "nc" not in _NC_CACHE:
        _NC_CACHE["nc"] = build_nc(d=d, s=s, b=b)
    nc = _NC_CACHE["nc"]
    in_maps = make_core_inputs(x, W_qkv, W_o, d=d, s=s, b=b)
    res = run_bass_kernel_spmd(nc, in_maps, core_ids=list(range(NCORES)))
    out = res.results[0]["out"].astype(np.float64)
    for c in range(1, NCORES):
        out += res.results[c]["out"]
    return out.astype(np.float32).reshape(b, s, d)



# revision 3
# speedup vs baseline: 1.0464x; 1.0464x over previous
"""Causal multi-head self-attention on 8 Trainium2 NeuronCores.

Sharding: tensor-parallel over heads. 16 heads / 8 cores = 2 heads per core.
Each core computes the QKV projection for its 2 heads (full sequence, both
batches), causal flash-style attention for its 2 heads, and a partial output
projection against its slice of W_o columns. The host sums the 8 partial
outputs (the "all-reduce" of the tensor-parallel scheme, done during unshard).

Matmul inputs are fp16 (PE streams 1 row/cycle vs 4 for fp32; fp16 keeps
11 mantissa bits vs bf16's 8), accumulation is always fp32 in PSUM, softmax
runs in fp32. End-to-end error vs the fp32 reference is ~5e-4 relative.

Device layout (contraction dim always on partitions):
  - x passed pre-transposed and pre-cast: xT [D, B*S] fp16.
  - Projection computes Q^T/K^T/V^T [128=2*dk, S] per batch directly.
  - Scores computed transposed, S^T[k, q] = K^T.T @ Q^T (fp32 PSUM), both
    heads into one [128, 2, 512] PSUM tile via separate PE row groups (the
    two matmuls run concurrently in different PE row strips).
  - One ACT exp per score tile (PSUM -> SBUF fp16), causal diagonal blocks
    column-sliced, the remaining 128-band masked with a triangular multiply.
  - V^T transposed on-PE to V[tok, dv] with a ones column appended, so the
    AV matmul also accumulates the softmax row-sums (row 64 of the output).
  - Normalization: stage O^T/row-sum to SBUF (frees PSUM), GPSIMD
    partition-broadcast of the row-sum (base-0 output only: HW ignores the
    out AP base), ~51ULP reciprocal, DVE multiply into mhaT fp16.
  - Output projection: out[tok,:] = mhaT_tile.T @ WoT, fp16 result to DRAM
    (host accumulates partials in float64).

Scheduling is built around the PE HAM clock gate (PE runs 1.2 GHz cold /
2.4 GHz warm; ~3.4us activity windows flip the state):
  - Dummy warm-up matmuls (no DMA deps) run under the initial input DMA so
    the PE is busy from t=0 and the first real matmul starts warm.
  - Input DMAs are issued finest-needed-first across all four issue queues:
    W_q pieces + the first 512 tokens of x(batch 0) land first so the first
    projection group starts ~4us in instead of ~18us.
  - Batch 1's projection groups + V transposes are interleaved as
    fine-grained fillers into batch 0's attention (which is exp/ACT-gated),
    so the PE never idles long enough to re-throttle at the batch boundary.
  - Batch 0's output projection drains inside batch 0/1 attention; only
    batch 1's last-chunk projection remains as a short tail.
"""

import numpy as np

import concourse.bacc as bacc
import concourse.mybir as mybir
import concourse.tile as tile

FP32 = mybir.dt.float32
FP16 = mybir.dt.float16

B = 2
S = 2048
D = 1024
NUM_HEADS = 16
DK = 64
NCORES = 8
HPC = NUM_HEADS // NCORES  # heads per core = 2
HD = HPC * DK  # 128, head dims per core

QCW = 512  # q chunk width
KTW = 128  # k tile width (partition dim)

WARMUP_MM = 12  # dummy PE matmuls issued under the startup DMA

NP_IN = np.float16


def build_nc(d=D, s=S, b=B):
    """Build the per-core Bass program. All 8 cores run this same program."""
    assert d % 128 == 0 and s % QCW == 0 and QCW % KTW == 0
    ndc = d // 128  # d_model chunks
    nqc = s // QCW  # q chunks per batch
    nkt = s // KTW  # k tiles per batch
    kpq = QCW // KTW  # k tiles per q chunk (4)
    ntt = s // 128  # token tiles per batch

    nc = bacc.Bacc("TRN2", target_bir_lowering=False)

    xT_d = nc.dram_tensor("xT", [d, b * s], FP16, kind="ExternalInput")
    # wqkvT packed host-side as [3, ndc, 128, 128] so each (m, k) piece is
    # a contiguous 32KB DRAM block (dense DMA descriptors).
    wt_d = nc.dram_tensor("wqkvT", [3 * ndc * 128, 128], FP16, kind="ExternalInput")
    wo_d = nc.dram_tensor("woT", [HD, d], FP16, kind="ExternalInput")
    tri_d = nc.dram_tensor("tri", [128, 128], FP16, kind="ExternalInput")
    id_d = nc.dram_tensor("ident", [128, 128], FP16, kind="ExternalInput")
    out_d = nc.dram_tensor("out", [b * s, d], FP16, kind="ExternalOutput")

    with tile.TileContext(nc) as tc:
        with (
            tc.tile_pool(name="consts", bufs=1) as consts,
            tc.tile_pool(name="xts", bufs=b * ndc) as xts_pool,
            tc.tile_pool(name="qkv", bufs=2) as qkv_pool,
            tc.tile_pool(name="vsb", bufs=2) as v_pool,
            tc.tile_pool(name="pt", bufs=4) as pt_pool,
            tc.tile_pool(name="mha", bufs=2) as mha_pool,
            tc.tile_pool(name="osb", bufs=3) as out_pool,
            tc.tile_pool(name="small", bufs=2) as small_pool,
            tc.tile_pool(name="ps_mm", bufs=2, space="PSUM") as ps_mm,
            tc.tile_pool(name="ps_s", bufs=2, space="PSUM") as ps_s,
            tc.tile_pool(name="ps_o", bufs=1, space="PSUM") as ps_o,
        ):
            # ---- PE warm-up fodder: zero tile with no DMA dependency.
            warm_sb = consts.tile([128, QCW], FP16)
            nc.gpsimd.memset(warm_sb, 0.0)

            # ---- input loads, finest-needed-first, round-robin over all
            # four DMA-issue queues. Wave 1 covers the first projection
            # group (W_q pieces + x[b0, tokens 0:512]); the rest streams in
            # behind it.
            dmae = [nc.sync, nc.scalar, nc.gpsimd]
            dma_i = [0]

            def dma(dst, src):
                eng = dmae[dma_i[0] % len(dmae)]
                dma_i[0] += 1
                eng.dma_start(dst, src)

            wt_sb = consts.tile([128, ndc, 3 * HD], FP16)

            def load_wt(m, k):
                r0 = (m * ndc + k) * 128
                dma(wt_sb[:, k, 128 * m : 128 * (m + 1)], wt_d[r0 : r0 + 128, :])

            xts_all = []
            for bi in range(b):
                xts_all.append(
                    [
                        xts_pool.tile([128, s], FP16, name=f"xt{bi}_{k}", tag="xt")
                        for k in range(ndc)
                    ]
                )
            # wave 1: first proj group's deps
            for k in range(ndc):
                load_wt(0, k)
            for k in range(ndc):
                dma(
                    xts_all[0][k][:, 0:QCW],
                    xT_d[128 * k : 128 * (k + 1), 0:QCW],
                )
            # wave 2: K/V weights, attention constants
            for m in range(1, 3):
                for k in range(ndc):
                    load_wt(m, k)
            tri_sb = consts.tile([128, 128], FP16)
            dma(tri_sb, tri_d[:, :])
            id_sb = consts.tile([128, 128], FP16)
            dma(id_sb, id_d[:, :])
            # wave 3: rest of batch 0 x, n-major so proj groups unblock in
            # emission order
            for n in range(1, nqc):
                for k in range(ndc):
                    dma(
                        xts_all[0][k][:, QCW * n : QCW * (n + 1)],
                        xT_d[128 * k : 128 * (k + 1), QCW * n : QCW * (n + 1)],
                    )
            wo_sb = consts.tile([128, d], FP16)
            dma(wo_sb, wo_d[:, :])
            # wave 4: batch 1 x
            for bi in range(1, b):
                for k in range(ndc):
                    dma(
                        xts_all[bi][k],
                        xT_d[128 * k : 128 * (k + 1), bi * s : (bi + 1) * s],
                    )

            # ---- dummy matmuls: keep the PE busy (HAM warm-up) while wave 1
            # lands. Results are never read.
            for _ in range(WARMUP_MM):
                wp = ps_mm.tile([128, QCW], FP32, name="wp", tag="mm")
                nc.tensor.matmul(wp, warm_sb[:, 0:128], warm_sb)
            # preload the ACT exp table so the first real exp doesn't pay it
            warm_exp = consts.tile([1, 1], FP16)
            nc.scalar.activation(
                warm_exp, warm_sb[0:1, 0:1], mybir.ActivationFunctionType.Exp
            )

            qkvTs = [qkv_pool.tile([128, 3, s], FP16, name=f"qkvT{bi}", tag="qkvT")
                     for bi in range(b)]
            v_sbs = [v_pool.tile([128, nkt, 2 * (DK + 1)], FP16, name=f"v{bi}",
                                 tag="vsb") for bi in range(b)]
            mhaTs = [mha_pool.tile([128, s], FP16, name=f"mhaT{bi}", tag="mhaT")
                     for bi in range(b)]

            def emit_proj_group(bi, m, n, evac_vec=False):
                qkvT, xts = qkvTs[bi], xts_all[bi]
                pp = ps_mm.tile([128, QCW], FP32, name="pp", tag="mm")
                for k in range(ndc):
                    nc.tensor.matmul(
                        pp,
                        wt_sb[:, k, 128 * m : 128 * (m + 1)],
                        xts[k][:, QCW * n : QCW * (n + 1)],
                        start=(k == 0),
                        stop=(k == ndc - 1),
                    )
                # PSUM fp32 -> fp16 SBUF on the way out. ACT is idle during
                # the dedicated projection phase; interleaved (filler)
                # groups evacuate on DVE instead so ACT can keep exp-ing.
                if evac_vec:
                    nc.vector.tensor_copy(
                        qkvT[:, m, QCW * n : QCW * (n + 1)], pp
                    )
                else:
                    nc.scalar.copy(qkvT[:, m, QCW * n : QCW * (n + 1)], pp)

            def emit_vsb_init(bi):
                nc.gpsimd.memset(v_sbs[bi], 1.0)

            def emit_trans(bi, t):
                qkvT, v_sb = qkvTs[bi], v_sbs[bi]
                tp = ps_mm.tile([128, 128], FP16, name="tp", tag="mm")
                nc.tensor.transpose(tp, qkvT[:, 2, 128 * t : 128 * (t + 1)], id_sb)
                nc.vector.tensor_copy(v_sb[:, t, 0:DK], tp[:, 0:DK])
                nc.vector.tensor_copy(
                    v_sb[:, t, DK + 1 : 2 * DK + 1], tp[:, DK : 2 * DK]
                )

            def emit_attn_chunk(bi, qc, fillers, rate=1):
                """One q-chunk of attention for batch bi.

                The AV matmuls lag the score/exp stream by LAG blocks so the
                first AV (which must wait for the previous chunk's PSUM
                accumulator to free) never head-of-line-blocks the in-order
                PE queue that feeds ACT. `fillers` (independent PE work:
                out-projection tiles, the other batch's projection pieces)
                drain up to `rate` per block in the lagged slot.
                """
                qkvT, v_sb, mhaT = qkvTs[bi], v_sbs[bi], mhaTs[bi]
                q0 = QCW * qc
                oA = ps_o.tile([DK + 1, QCW], FP32, name="oA", tag="oA")
                oB = ps_o.tile([DK + 1, QCW], FP32, name="oB", tag="oB")
                kts = kpq * (qc + 1)
                LAG = 2
                pts = {}
                for i in range(kts + LAG):
                    if i < kts:
                        kt = i
                        c0 = KTW * (kt - kpq * qc) if kt >= kpq * qc else 0
                        sp = ps_s.tile([128, 2, QCW], FP32, name="sp", tag="s")
                        # scores S^T[k, q]; heads in separate PE row groups
                        nc.tensor.matmul(
                            sp[:, 0, c0:QCW],
                            qkvT[0:DK, 1, KTW * kt : KTW * (kt + 1)],
                            qkvT[0:DK, 0, q0 + c0 : q0 + QCW],
                        )
                        nc.tensor.matmul(
                            sp[:, 1, c0:QCW],
                            qkvT[DK : 2 * DK, 1, KTW * kt : KTW * (kt + 1)],
                            qkvT[DK : 2 * DK, 0, q0 + c0 : q0 + QCW],
                        )
                        pt = pt_pool.tile(
                            [128, 2, QCW], FP16, name="pt", tag="pt"
                        )
                        nc.scalar.activation(
                            pt[:, :, c0:QCW],
                            sp[:, :, c0:QCW],
                            mybir.ActivationFunctionType.Exp,
                        )
                        if kt >= kpq * qc:
                            # triangular mask on the diagonal 128-band
                            nc.vector.tensor_mul(
                                pt[:, 0, c0 : c0 + KTW],
                                pt[:, 0, c0 : c0 + KTW],
                                tri_sb,
                            )
                            nc.vector.tensor_mul(
                                pt[:, 1, c0 : c0 + KTW],
                                pt[:, 1, c0 : c0 + KTW],
                                tri_sb,
                            )
                        pts[kt] = (pt, c0)
                    if i >= LAG:
                        kt = i - LAG
                        pt, c0 = pts.pop(kt)
                        nc.tensor.matmul(
                            oA[:, c0:QCW],
                            v_sb[:, kt, 0 : DK + 1],
                            pt[:, 0, c0:QCW],
                            start=(kt == 0),
                            stop=(kt == kts - 1),
                        )
                        nc.tensor.matmul(
                            oB[:, c0:QCW],
                            v_sb[:, kt, DK + 1 : 2 * DK + 2],
                            pt[:, 1, c0:QCW],
                            start=(kt == 0),
                            stop=(kt == kts - 1),
                        )
                        for _ in range(rate):
                            if fillers:
                                fillers.pop(0)()
                # normalize: stage O^T + row-sum to base-0 SBUF (frees PSUM),
                # broadcast row-sum (base-0 out only), reciprocal, multiply
                for h, oh in ((0, oA), (1, oB)):
                    ost = small_pool.tile([DK, QCW], FP32, name="ost", tag=f"ost{h}")
                    nc.vector.tensor_copy(ost, oh[0:DK, :])
                    t = small_pool.tile([1, QCW], FP32, name="t", tag=f"t{h}")
                    nc.vector.tensor_copy(t, oh[DK : DK + 1, :])
                    bc = small_pool.tile([DK, QCW], FP32, name="bc", tag=f"bc{h}")
                    nc.gpsimd.partition_broadcast(bc, t, channels=DK)
                    nc.vector.reciprocal_approx_fast(out=bc, in_=bc)
                    nc.vector.tensor_mul(
                        mhaT[DK * h : DK * (h + 1), q0 : q0 + QCW], ost, bc
                    )

            def emit_fp_tile(bi, t):
                mhaT = mhaTs[bi]
                fps = []
                for half in range(d // QCW):
                    fp = ps_mm.tile([128, QCW], FP32, name="fp", tag="mm")
                    nc.tensor.matmul(
                        fp,
                        mhaT[:, 128 * t : 128 * (t + 1)],
                        wo_sb[:, QCW * half : QCW * (half + 1)],
                    )
                    fps.append(fp)
                ob = out_pool.tile([128, d], FP16, name="ob", tag="ob")
                for half in range(d // QCW):
                    nc.vector.tensor_copy(
                        ob[:, QCW * half : QCW * (half + 1)], fps[half]
                    )
                r0 = bi * s + 128 * t
                nc.sync.dma_start(out_d[r0 : r0 + 128, :], ob)

            tpq = ntt // nqc  # out-proj token tiles ready per q-chunk

            # ---- batch 0 projection + V transposes (PE-dense phase).
            # n-outer matches the x DMA arrival order so the PE never waits
            # on a load that comes later.
            for n in range(nqc):
                for m in range(3):
                    emit_proj_group(0, m, n)
            emit_vsb_init(0)
            for t in range(nkt):
                emit_trans(0, t)

            # ---- batch 0 attention. Fillers: batch 1's entire projection
            # (fine-grained: one group or transpose per slot) plus batch 0's
            # out-projection tiles as their chunks normalize.
            fillers = []
            for n in range(nqc):
                for m in range(3):
                    fillers.append(
                        lambda m=m, n=n: emit_proj_group(1, m, n, evac_vec=True)
                    )
            fillers.append(lambda: emit_vsb_init(1))
            for t in range(nkt):
                fillers.append(lambda t=t: emit_trans(1, t))

            for qc in range(nqc):
                emit_attn_chunk(0, qc, fillers, rate=1)
                for t in range(tpq * qc, tpq * (qc + 1)):
                    fillers.append(lambda t=t: emit_fp_tile(0, t))
            # drain any fillers the chunk slots didn't cover (keeps PE dense
            # across the batch boundary)
            rest, fillers = fillers, []
            for th in rest:
                th()

            # ---- batch 1 attention. Fillers: batch 1's own out-projection
            # tiles, one chunk behind.
            prev_fp = []
            for qc in range(nqc):
                emit_attn_chunk(1, qc, prev_fp, rate=1)
                prev_fp = [
                    (lambda t=t: emit_fp_tile(1, t))
                    for t in range(tpq * qc, tpq * (qc + 1))
                ]
            for th in prev_fp:
                th()

    nc.compile()
    return nc


def make_core_inputs(x, W_qkv, W_o, d=D, s=S, b=B):
    """Host-side shard prep. Returns list of per-core input dicts."""
    nh = W_qkv.shape[0] // (3 * DK)
    ndc = d // 128
    xT = np.ascontiguousarray(
        x.astype(np.float32).transpose(2, 0, 1).reshape(d, b * s).astype(NP_IN)
    )
    tri = np.triu(np.ones((128, 128), dtype=NP_IN))  # tri[k,q]=1 iff q>=k
    ident = np.eye(128, dtype=NP_IN)
    scale = np.float32(1.0 / np.sqrt(DK))
    in_maps = []
    for c in range(NCORES):
        h0 = HPC * c
        r = slice(h0 * DK, (h0 + HPC) * DK)
        wq = W_qkv[0 * nh * DK :][r] * scale
        wk = W_qkv[1 * nh * DK :][r]
        wv = W_qkv[2 * nh * DK :][r]
        ws = np.concatenate([wq, wk, wv], axis=0)  # [3*HD, d]
        wT = np.ascontiguousarray(ws.T.astype(NP_IN))  # [d, 3*HD]
        # repack into [3, ndc, 128, 128] so each (m, k) piece is contiguous
        wTp = np.ascontiguousarray(
            wT.reshape(ndc, 128, 3, 128).transpose(2, 0, 1, 3).reshape(-1, 128)
        )
        woT = np.ascontiguousarray(W_o[:, r].T.astype(NP_IN))  # [HD, d]
        in_maps.append(
            {"xT": xT, "wqkvT": wTp, "woT": woT, "tri": tri, "ident": ident}
        )
    return in_maps


_NC_CACHE = {}


def kernel(x, W_qkv, W_o):
    from concourse.bass_utils import run_bass_kernel_spmd

    b, s, d = x.shape
    if "nc" not in _NC_CACHE:
        _NC_CACHE["nc"] = build_nc(d=d, s=s, b=b)
    nc = _NC_CACHE["nc"]
    in_maps = make_core_inputs(x, W_qkv, W_o, d=d, s=s, b=b)
    res = run_bass_kernel_spmd(nc, in_maps, core_ids=list(range(NCORES)))
    out = res.results[0]["out"].astype(np.float64)
    for c in range(1, NCORES):
        out += res.results[c]["out"]
    return out.astype(np.float32).reshape(b, s, d)


# revision 12
# speedup vs baseline: 1.1176x; 1.0680x over previous
"""Causal multi-head self-attention on 8 Trainium2 NeuronCores.

Sharding: tensor-parallel over heads. 16 heads / 8 cores = 2 heads per core.
Each core computes the QKV projection for its 2 heads (full sequence, both
batches), causal flash-style attention for its 2 heads, and a partial output
projection against its slice of W_o columns. The host sums the 8 partial
outputs (the "all-reduce" of the tensor-parallel scheme, done during unshard).

Matmul inputs are fp16 (PE streams 1 row/cycle vs 4 for fp32; fp16 keeps
11 mantissa bits vs bf16's 8), accumulation is always fp32 in PSUM, softmax
runs in fp32. End-to-end error vs the fp32 reference is ~5e-4 relative.

Device layout (contraction dim always on partitions):
  - x passed pre-transposed and pre-cast: xT [D, B*S] fp16.
  - Projection computes Q^T/K^T/V^T [128=2*dk, S] per batch directly.
  - Scores computed transposed, S^T[k, q] = K^T.T @ Q^T (fp32 PSUM), both
    heads into one [128, 2, 512] PSUM tile via separate PE row groups (the
    two matmuls run concurrently in different PE row strips).
  - One ACT exp per score tile (PSUM -> SBUF fp16), causal diagonal blocks
    column-sliced, the remaining 128-band masked with a triangular multiply.
  - V^T transposed on-PE to V[tok, dv] with a ones column appended, so the
    AV matmul also accumulates the softmax row-sums (row 64 of the output).
  - Normalization: stage O^T/row-sum to SBUF (frees PSUM), GPSIMD
    partition-broadcast of the row-sum (base-0 output only: HW ignores the
    out AP base), ~51ULP reciprocal, DVE multiply into mhaT fp16.
  - Output projection: out[tok,:] = mhaT_tile.T @ WoT, fp16 result to DRAM
    (host accumulates partials in float64).

Scheduling is built around the PE HAM clock gate (PE runs 1.2 GHz cold /
2.4 GHz warm; ~3.4us activity windows flip the state):
  - Dummy warm-up matmuls (no DMA deps) run under the initial input DMA so
    the PE is busy from t=0 and the first real matmul starts warm.
  - Input DMAs are issued finest-needed-first across all four issue queues:
    W_q pieces + the first 512 tokens of x(batch 0) land first so the first
    projection group starts ~4us in instead of ~18us.
  - Batch 1's projection groups + V transposes are interleaved as
    fine-grained fillers into batch 0's attention (which is exp/ACT-gated),
    so the PE never idles long enough to re-throttle at the batch boundary.
  - Batch 0's output projection drains inside batch 0/1 attention; only
    batch 1's last-chunk projection remains as a short tail.
"""

import numpy as np

import concourse.bacc as bacc
import concourse.mybir as mybir
import concourse.tile as tile

FP32 = mybir.dt.float32
FP16 = mybir.dt.float16

B = 2
S = 2048
D = 1024
NUM_HEADS = 16
DK = 64
NCORES = 8
HPC = NUM_HEADS // NCORES  # heads per core = 2
HD = HPC * DK  # 128, head dims per core

QCW = 512  # q chunk width
KTW = 128  # k tile width (partition dim)

WARMUP_MM = 14  # dummy PE matmuls issued under the startup DMA

NP_IN = np.float16


def build_nc(d=D, s=S, b=B):
    """Build the per-core Bass program. All 8 cores run this same program."""
    assert d % 128 == 0 and s % QCW == 0 and QCW % KTW == 0
    ndc = d // 128  # d_model chunks
    nqc = s // QCW  # q chunks per batch
    nkt = s // KTW  # k tiles per batch
    kpq = QCW // KTW  # k tiles per q chunk (4)
    ntt = s // 128  # token tiles per batch

    nc = bacc.Bacc("TRN2", target_bir_lowering=False)

    xT_d = nc.dram_tensor("xT", [d, b * s], FP16, kind="ExternalInput")
    # wqkvT packed host-side as [3, ndc, 128, 128] so each (m, k) piece is
    # a contiguous 32KB DRAM block (dense DMA descriptors).
    wt_d = nc.dram_tensor("wqkvT", [3 * ndc * 128, 128], FP16, kind="ExternalInput")
    wo_d = nc.dram_tensor("woT", [HD, d], FP16, kind="ExternalInput")
    tri_d = nc.dram_tensor("tri", [128, 128], FP16, kind="ExternalInput")
    id_d = nc.dram_tensor("ident", [128, 128], FP16, kind="ExternalInput")
    out_d = nc.dram_tensor("out", [b * s, d], FP16, kind="ExternalOutput")

    with tile.TileContext(nc) as tc:
        with (
            tc.tile_pool(name="consts", bufs=1) as consts,
            tc.tile_pool(name="xts", bufs=b * ndc) as xts_pool,
            tc.tile_pool(name="qkv", bufs=2) as qkv_pool,
            tc.tile_pool(name="vsb", bufs=2) as v_pool,
            tc.tile_pool(name="pt", bufs=4) as pt_pool,
            tc.tile_pool(name="mha", bufs=2) as mha_pool,
            tc.tile_pool(name="osb", bufs=3) as out_pool,
            tc.tile_pool(name="small", bufs=2) as small_pool,
            tc.tile_pool(name="ps_mm", bufs=2, space="PSUM") as ps_mm,
            tc.tile_pool(name="ps_s", bufs=2, space="PSUM") as ps_s,
            tc.tile_pool(name="ps_o", bufs=1, space="PSUM") as ps_o,
        ):
            # ---- PE warm-up fodder: zero tile with no DMA dependency.
            warm_sb = consts.tile([128, QCW], FP16)
            nc.gpsimd.memset(warm_sb, 0.0)

            # ---- input loads, finest-needed-first, round-robin over all
            # four DMA-issue queues. Wave 1 covers the first projection
            # group (W_q pieces + x[b0, tokens 0:512]); the rest streams in
            # behind it.
            dmae = [nc.sync, nc.scalar, nc.gpsimd]
            dma_i = [0]

            def dma(dst, src):
                eng = dmae[dma_i[0] % len(dmae)]
                dma_i[0] += 1
                eng.dma_start(dst, src)

            wt_sb = consts.tile([128, ndc, 3 * HD], FP16)

            def load_wt(m, k):
                r0 = (m * ndc + k) * 128
                dma(wt_sb[:, k, 128 * m : 128 * (m + 1)], wt_d[r0 : r0 + 128, :])

            xts_all = []
            for bi in range(b):
                xts_all.append(
                    [
                        xts_pool.tile([128, s], FP16, name=f"xt{bi}_{k}", tag="xt")
                        for k in range(ndc)
                    ]
                )
            # wave 1: weights first (small), then batch-0 x chunks in k order
            # (full [128, s] chunks keep 4KB per-partition DMA lines; the
            # k-accumulation of the first projection group pipelines behind
            # the arriving chunks). wt m1/m2 interleave with the first two x
            # chunks so every projection group unblocks in emission order.
            for k in range(ndc):
                load_wt(0, k)
            dma(xts_all[0][0], xT_d[0:128, 0:s])
            for k in range(ndc):
                load_wt(1, k)
            dma(xts_all[0][1], xT_d[128:256, 0:s])
            for k in range(ndc):
                load_wt(2, k)
            tri_sb = consts.tile([128, 128], FP16)
            dma(tri_sb, tri_d[:, :])
            id_sb = consts.tile([128, 128], FP16)
            dma(id_sb, id_d[:, :])
            for k in range(2, ndc):
                dma(xts_all[0][k], xT_d[128 * k : 128 * (k + 1), 0:s])
            wo_sb = consts.tile([128, d], FP16)
            dma(wo_sb, wo_d[:, :])
            # wave 2: batch 1 x
            for bi in range(1, b):
                for k in range(ndc):
                    dma(
                        xts_all[bi][k],
                        xT_d[128 * k : 128 * (k + 1), bi * s : (bi + 1) * s],
                    )

            # ---- dummy matmuls: keep the PE busy (HAM warm-up) while wave 1
            # lands. Results are never read.
            for _ in range(WARMUP_MM):
                wp = ps_mm.tile([128, QCW], FP32, name="wp", tag="mm")
                nc.tensor.matmul(wp, warm_sb[:, 0:128], warm_sb)
            # preload the ACT exp table so the first real exp doesn't pay it
            warm_exp = consts.tile([1, 1], FP16)
            nc.scalar.activation(
                warm_exp, warm_sb[0:1, 0:1], mybir.ActivationFunctionType.Exp
            )

            qkvTs = [qkv_pool.tile([128, 3, s], FP16, name=f"qkvT{bi}", tag="qkvT")
                     for bi in range(b)]
            v_sbs = [v_pool.tile([128, nkt, 2 * (DK + 1)], FP16, name=f"v{bi}",
                                 tag="vsb") for bi in range(b)]
            mhaTs = [mha_pool.tile([128, s], FP16, name=f"mhaT{bi}", tag="mhaT")
                     for bi in range(b)]

            def emit_proj_group(bi, m, n):
                qkvT, xts = qkvTs[bi], xts_all[bi]
                pp = ps_mm.tile([128, QCW], FP32, name="pp", tag="mm")
                for k in range(ndc):
                    nc.tensor.matmul(
                        pp,
                        wt_sb[:, k, 128 * m : 128 * (m + 1)],
                        xts[k][:, QCW * n : QCW * (n + 1)],
                        start=(k == 0),
                        stop=(k == ndc - 1),
                    )
                # ACT casts fp32 PSUM -> fp16 SBUF on the way out (it has
                # slack: exp of the previous chunk is smaller than the PE
                # work of this projection group + chunk).
                nc.scalar.copy(qkvT[:, m, QCW * n : QCW * (n + 1)], pp)

            def emit_vsb_init(bi):
                nc.gpsimd.memset(v_sbs[bi], 1.0)

            def emit_trans(bi, t):
                qkvT, v_sb = qkvTs[bi], v_sbs[bi]
                tp = ps_mm.tile([128, 128], FP16, name="tp", tag="mm")
                nc.tensor.transpose(tp, qkvT[:, 2, 128 * t : 128 * (t + 1)], id_sb)
                nc.vector.tensor_copy(v_sb[:, t, 0:DK], tp[:, 0:DK])
                nc.vector.tensor_copy(
                    v_sb[:, t, DK + 1 : 2 * DK + 1], tp[:, DK : 2 * DK]
                )

            def emit_attn_chunk(bi, qc, fillers, rate=1, pieces=1, tail=None):
                """One q-chunk of attention for batch bi.

                The AV matmuls lag the score/exp stream by LAG blocks so the
                first AV (which must wait for the previous chunk's PSUM
                accumulator to free) never head-of-line-blocks the in-order
                PE queue that feeds ACT. `fillers` (independent PE work:
                out-projection tiles, the other batch's projection pieces)
                drain up to `rate` per block in the lagged slot.
                """
                qkvT, v_sb, mhaT = qkvTs[bi], v_sbs[bi], mhaTs[bi]
                q0 = QCW * qc
                oA = ps_o.tile([DK + 1, QCW], FP32, name="oA", tag="oA")
                oB = ps_o.tile([DK + 1, QCW], FP32, name="oB", tag="oB")
                kts = kpq * (qc + 1)
                LAG = 2
                pts = {}
                for i in range(kts + LAG):
                    if i < kts:
                        kt = i
                        c0 = KTW * (kt - kpq * qc) if kt >= kpq * qc else 0
                        sp = ps_s.tile([128, 2, QCW], FP32, name="sp", tag="s")
                        # scores S^T[k, q]; heads in separate PE row groups
                        nc.tensor.matmul(
                            sp[:, 0, c0:QCW],
                            qkvT[0:DK, 1, KTW * kt : KTW * (kt + 1)],
                            qkvT[0:DK, 0, q0 + c0 : q0 + QCW],
                        )
                        nc.tensor.matmul(
                            sp[:, 1, c0:QCW],
                            qkvT[DK : 2 * DK, 1, KTW * kt : KTW * (kt + 1)],
                            qkvT[DK : 2 * DK, 0, q0 + c0 : q0 + QCW],
                        )
                        pt = pt_pool.tile(
                            [128, 2, QCW], FP16, name="pt", tag="pt"
                        )
                        nc.scalar.activation(
                            pt[:, :, c0:QCW],
                            sp[:, :, c0:QCW],
                            mybir.ActivationFunctionType.Exp,
                        )
                        if kt >= kpq * qc:
                            # triangular mask on the diagonal 128-band
                            nc.vector.tensor_mul(
                                pt[:, 0, c0 : c0 + KTW],
                                pt[:, 0, c0 : c0 + KTW],
                                tri_sb,
                            )
                            nc.vector.tensor_mul(
                                pt[:, 1, c0 : c0 + KTW],
                                pt[:, 1, c0 : c0 + KTW],
                                tri_sb,
                            )
                        pts[kt] = (pt, c0)
                    if i >= LAG:
                        kt = i - LAG
                        pt, c0 = pts.pop(kt)
                        nc.tensor.matmul(
                            oA[:, c0:QCW],
                            v_sb[:, kt, 0 : DK + 1],
                            pt[:, 0, c0:QCW],
                            start=(kt == 0),
                            stop=(kt == kts - 1),
                        )
                        nc.tensor.matmul(
                            oB[:, c0:QCW],
                            v_sb[:, kt, DK + 1 : 2 * DK + 2],
                            pt[:, 1, c0:QCW],
                            start=(kt == 0),
                            stop=(kt == kts - 1),
                        )
                        for _ in range(rate):
                            if fillers:
                                fillers.pop(0)()
                # normalize: stage O^T + row-sum to base-0 SBUF in one copy
                # (frees PSUM), broadcast row-sum (base-0 out only: HW
                # ignores the out AP base), reciprocal, multiply.
                # `pieces` > 1 splits the chain column-wise so the caller
                # can interleave out-projection tiles that each depend on
                # only one piece (used for the final chunk's tail).
                pw = QCW // pieces
                for p in range(pieces):
                    c = slice(pw * p, pw * (p + 1))
                    for h, oh in ((0, oA), (1, oB)):
                        ost = small_pool.tile(
                            [DK, pw], FP32, name="ost", tag=f"ost{h}{p}"
                        )
                        nc.vector.tensor_copy(ost, oh[0:DK, c])
                        t = small_pool.tile(
                            [1, pw], FP32, name="t", tag=f"t{h}{p}"
                        )
                        nc.vector.tensor_copy(t, oh[DK : DK + 1, c])
                        bc = small_pool.tile(
                            [DK, pw], FP32, name="bc", tag=f"bc{h}{p}"
                        )
                        nc.gpsimd.partition_broadcast(bc, t, channels=DK)
                        nc.vector.reciprocal_approx_fast(out=bc, in_=bc)
                        nc.vector.tensor_mul(
                            mhaT[DK * h : DK * (h + 1), q0 + pw * p : q0 + pw * (p + 1)],
                            ost,
                            bc,
                        )
                    if tail:
                        tail.pop(0)()

            def emit_fp_tile(bi, t):
                mhaT = mhaTs[bi]
                fps = []
                for half in range(d // QCW):
                    fp = ps_mm.tile([128, QCW], FP32, name="fp", tag="mm")
                    nc.tensor.matmul(
                        fp,
                        mhaT[:, 128 * t : 128 * (t + 1)],
                        wo_sb[:, QCW * half : QCW * (half + 1)],
                    )
                    fps.append(fp)
                ob = out_pool.tile([128, d], FP16, name="ob", tag="ob")
                for half in range(d // QCW):
                    nc.vector.tensor_copy(
                        ob[:, QCW * half : QCW * (half + 1)], fps[half]
                    )
                r0 = bi * s + 128 * t
                nc.sync.dma_start(out_d[r0 : r0 + 128, :], ob)

            tpq = ntt // nqc  # out-proj token tiles ready per q-chunk

            # ---- fused pipeline: per q-chunk, emit the chunk's projection
            # groups + V transposes (PE-dense), then the chunk's attention
            # (exp/ACT-gated, PE slack filled with the previous chunk's
            # out-projection tiles). The projection of chunk n overlaps the
            # exp stream of chunk n-1 on ACT, so neither engine idles long
            # enough to re-throttle the PE clock. The final chunk normalizes
            # in 128-column pieces with its out-projection tiles pipelined
            # in, shrinking the end-of-kernel tail.
            fillers = []
            for bi in range(b):
                emit_vsb_init(bi)
                for n in range(nqc):
                    for m in range(3):
                        emit_proj_group(bi, m, n)
                    for t in range(kpq * n, kpq * (n + 1)):
                        emit_trans(bi, t)
                    last = bi == b - 1 and n == nqc - 1
                    if not last:
                        emit_attn_chunk(bi, n, fillers, rate=1)
                        fillers += [
                            (lambda bi=bi, t=t: emit_fp_tile(bi, t))
                            for t in range(tpq * n, tpq * (n + 1))
                        ]
                    else:
                        tail = [
                            (lambda t=t: emit_fp_tile(1, t))
                            for t in range(tpq * n, tpq * (n + 1))
                        ]
                        emit_attn_chunk(
                            1, n, fillers, rate=1, pieces=tpq, tail=tail
                        )
            for th in fillers:
                th()

    nc.compile()
    return nc


def make_core_inputs(x, W_qkv, W_o, d=D, s=S, b=B):
    """Host-side shard prep. Returns list of per-core input dicts."""
    nh = W_qkv.shape[0] // (3 * DK)
    ndc = d // 128
    xT = np.ascontiguousarray(
        x.astype(np.float32).transpose(2, 0, 1).reshape(d, b * s).astype(NP_IN)
    )
    tri = np.triu(np.ones((128, 128), dtype=NP_IN))  # tri[k,q]=1 iff q>=k
    ident = np.eye(128, dtype=NP_IN)
    scale = np.float32(1.0 / np.sqrt(DK))
    in_maps = []
    for c in range(NCORES):
        h0 = HPC * c
        r = slice(h0 * DK, (h0 + HPC) * DK)
        wq = W_qkv[0 * nh * DK :][r] * scale
        wk = W_qkv[1 * nh * DK :][r]
        wv = W_qkv[2 * nh * DK :][r]
        ws = np.concatenate([wq, wk, wv], axis=0)  # [3*HD, d]
        wT = np.ascontiguousarray(ws.T.astype(NP_IN))  # [d, 3*HD]
        # repack into [3, ndc, 128, 128] so each (m, k) piece is contiguous
        wTp = np.ascontiguousarray(
            wT.reshape(ndc, 128, 3, 128).transpose(2, 0, 1, 3).reshape(-1, 128)
        )
        woT = np.ascontiguousarray(W_o[:, r].T.astype(NP_IN))  # [HD, d]
        in_maps.append(
            {"xT": xT, "wqkvT": wTp, "woT": woT, "tri": tri, "ident": ident}
        )
    return in_maps


_NC_CACHE = {}


def kernel(x, W_qkv, W_o):
    from concourse.bass_utils import run_bass_kernel_spmd

    b, s, d = x.shape
    if "nc" not in _NC_CACHE:
        _NC_CACHE["nc"] = build_nc(d=d, s=s, b=b)
    nc = _NC_CACHE["nc"]
    in_maps = make_core_inputs(x, W_qkv, W_o, d=d, s=s, b=b)
    res = run_bass_kernel_spmd(nc, in_maps, core_ids=list(range(NCORES)))
    out = res.results[0]["out"].astype(np.float64)
    for c in range(1, NCORES):
        out += res.results[c]["out"]
    return out.astype(np.float32).reshape(b, s, d)
